# revision 1
# baseline (speedup 1.0000x reference)
"""Trainium2 Bass kernel for nn_ComplexConv2Deffangle4Dxy.

Reference math (per batch b, branch br):
    out[br] = pointwise(w2, depthwise3x3(w1, img[br]))   with zero padding P=1
      br=0 (rot): weights (w1n, w2n) where wn = (wx+wy)^2 / sum((wx+wy)^2)
      br=1 (abs): log-domain: exp(branch(log(img + EPS), w1n, w2n))
      br=2 (x):   weights (w1x, w2x)
      br=3 (y):   weights (w1y, w2y)

Kernel strategy (per NeuronCore, data-parallel over batch B=8 -> 8 cores):
  Fuse depthwise+pointwise into a single 3x3 conv whose weights are the
  outer product  Wf[o, c, k] = w2[o, c] * w1[c, k], computed as
  PSUM-accumulated matmuls over the 9 kernel offsets with
  lhsT = fused weights (K=Cin, M=Cout=128) and rhs = shifted image views.
  Images are zero-padded on the host (pure marshaling) so every shifted
  view is a plain strided AP with no boundary special cases; for the abs
  branch Ln(x*1+EPS) maps the zero padding to log(EPS), exactly matching
  the reference's pad-then-log order.  Weight normalization for the
  rot/abs branches is computed on device (sum via ones-matmul, reciprocal
  on DVE, scale folded into the fused conv weights).

  Scheme "dual": SBUF partitions 0..63 hold the padded image (A), 64..127
  hold the same image shifted down 2 rows (B).  A K=128 matmul at row
  offset r then contracts offset (dh=-1,dw) on the lower half and
  (dh=+1,dw) on the upper half in one instruction.  The dh=0 offsets use
  K=64 matmuls, pairwise packed onto disjoint PE row groups.

  Scheme "hsplit": partitions 0..63 serve output rows 0..31, partitions
  64..127 serve rows 32..63; all matmuls are K=64, issued in pairs on
  disjoint PE row groups (tile_position row tiling).
"""

import sys

for _p in ("/opt/trn_rl_repo",):
    if _p not in sys.path:
        sys.path.insert(0, _p)

import ml_dtypes
import numpy as np

import concourse.bacc as bacc
import concourse.mybir as mybir
import concourse.tile as tile
from concourse import bass_utils

F32 = mybir.dt.float32
F32R = mybir.dt.float32r
BF16 = mybir.dt.bfloat16

EPS = 1e-6
N_CORES = 8
B, NBR, CIN, COUT, H, W = 8, 4, 64, 128, 64, 64
HP, WP = H + 2, W + 2          # host-padded image
HS_ROWS = 35                   # hsplit: padded rows per partition half

# matmul input dtype: "f32r" | "f32" | "bf16"
MM_DTYPE = "f32r"
SCHEME = "hsplit"              # "dual" | "hsplit"
# Packing (0,+1) onto the upper PE row group (K=64 at base_partition 64)
# mixed with K=128 matmuls in the same PSUM accumulation group crashes at
# runtime on TRN2 hardware -- keep disabled.
DH0_UPPER_PACK = False
LOOP_ITERS = None              # benchmarking: device-side repeat count
PROBE = ""                     # "" | "no_out" (skip evac+out-DMA) | "no_mm"
TRACE = False
LAST_EXEC_TIME_NS = None
LAST_RESULTS = None

_PROG_CACHE = {}

# walrus's LDWEIGHTS optimization (split weight loads from matmuls so they
# pipeline through the PE reorder window) is hardcoded off in
# bass_utils.bir_verify_and_optimise; expose a switch that rewrites the flag
# inside the compile command.
LDW_OPT = False
_orig_run_command = bass_utils.run_command


def _patched_run_command(cmd, *a, **kw):
    if LDW_OPT and isinstance(cmd, list) and "--enable-ldw-opt=false" in cmd:
        cmd = ["--enable-ldw-opt=true" if c == "--enable-ldw-opt=false" else c for c in cmd]
    return _orig_run_command(cmd, *a, **kw)


bass_utils.run_command = _patched_run_command
if getattr(bass_utils, "bir_verify_and_optimise", None) is not None:
    bass_utils.bir_verify_and_optimise.__globals__["run_command"] = _patched_run_command

BRANCHES = (  # (branch index, weight set, log-domain?, evac engine)
    (2, "x", False, "v"),
    (3, "y", False, "a"),
    (0, "n", False, "v"),
    (1, "n", True, "a"),
)


def _mm_dt():
    return {"f32r": F32R, "f32": F32, "bf16": BF16}[MM_DTYPE]


def _np_in_dt():
    return ml_dtypes.bfloat16 if MM_DTYPE == "bf16" else np.float32


def _emit(nc, tc, xin_d, w1x_d, w1y_d, w2xT_d, w2yT_d, out_d):
    mdt = _mm_dt()
    img_rows = HP if SCHEME == "dual" else HS_ROWS
    with (
        tc.tile_pool(name="wp", bufs=1) as wp,
        tc.tile_pool(name="imgp", bufs=2) as imgp,
        tc.tile_pool(name="psr", bufs=2, space="PSUM") as psr,
        tc.tile_pool(name="psp", bufs=6, space="PSUM") as psp,
        tc.tile_pool(name="obp", bufs=6) as obp,
    ):
        # ---- weight prep -------------------------------------------------
        # All weight/source tiles replicated into both partition halves so
        # per-half fused tiles can be built with partition-local DVE ops.
        w1x_s = wp.tile([2 * CIN, 9], F32, tag="w1x")
        w1y_s = wp.tile([2 * CIN, 9], F32, tag="w1y")
        w2xT_s = wp.tile([2 * CIN, COUT], F32, tag="w2xT")
        w2yT_s = wp.tile([2 * CIN, COUT], F32, tag="w2yT")
        for t, d in (
            (w1x_s, w1x_d),
            (w1y_s, w1y_d),
            (w2xT_s, w2xT_d),
            (w2yT_s, w2yT_d),
        ):
            nc.sync.dma_start(out=t[0:CIN], in_=d)
            nc.sync.dma_start(out=t[CIN : 2 * CIN], in_=d)

        ones_k = wp.tile([CIN, 1], F32, tag="ones_k")
        nc.vector.memset(ones_k[:, :], 1.0)
        ones_m = wp.tile([1, 2 * CIN], F32, tag="ones_m")
        nc.vector.memset(ones_m[:, :], 1.0)
        eps_b = wp.tile([2 * CIN, 1], F32, tag="eps_b")
        nc.vector.memset(eps_b[:, :], float(EPS))
        zero_b = wp.tile([COUT, 1], F32, tag="zero_b")
        nc.vector.memset(zero_b[:, :], 0.0)

        # u1 = (w1x + w1y)^2, u2T = ((w2x + w2y)^2)^T  (both partition halves)
        u1 = wp.tile([2 * CIN, 9], F32, tag="u1")
        nc.vector.tensor_add(u1[:, :], w1x_s[:, :], w1y_s[:, :])
        nc.vector.tensor_mul(u1[:, :], u1[:, :], u1[:, :])
        u2T = wp.tile([2 * CIN, COUT], F32, tag="u2T")
        nc.vector.tensor_add(u2T[:, :], w2xT_s[:, :], w2yT_s[:, :])
        nc.vector.tensor_mul(u2T[:, :], u2T[:, :], u2T[:, :])

        # S1 = sum(u1), S2 = sum(u2) via ones-matmul + free-dim reduce
        s1v = psr.tile([1, 9], F32, tag="red")
        nc.tensor.matmul(s1v[:, :], ones_k[:, :], u1[0:CIN, :], start=True, stop=True)
        s2v = psr.tile([1, COUT], F32, tag="red")
        nc.tensor.matmul(s2v[:, :], ones_k[:, :], u2T[0:CIN, :], start=True, stop=True)
        s1 = wp.tile([1, 1], F32, tag="s1")
        nc.vector.tensor_reduce(
            s1[:, :], s1v[:, :], axis=mybir.AxisListType.X, op=mybir.AluOpType.add
        )
        s2 = wp.tile([1, 1], F32, tag="s2")
        nc.vector.tensor_reduce(
            s2[:, :], s2v[:, :], axis=mybir.AxisListType.X, op=mybir.AluOpType.add
        )
        inv = wp.tile([1, 1], F32, tag="inv")
        nc.vector.tensor_mul(inv[:, :], s1[:, :], s2[:, :])
        nc.vector.reciprocal(inv[:, :], inv[:, :])
        # broadcast 1/(S1*S2) to all 128 partitions
        invb_ps = psr.tile([2 * CIN, 1], F32, tag="red")
        nc.tensor.matmul(invb_ps[:, :], ones_m[:, :], inv[:, :], start=True, stop=True)
        invb = wp.tile([2 * CIN, 1], F32, tag="invb")
        nc.vector.tensor_copy(invb[:, :], invb_ps[:, :])
        # u2T_n = u2T / (S1*S2): both normalizations in one fold
        u2Tn = wp.tile([2 * CIN, COUT], F32, tag="u2Tn")
        nc.vector.tensor_scalar(
            u2Tn[:, :], u2T[:, :], invb[:, 0:1], None, mybir.AluOpType.mult
        )

        # fused weight tiles
        #  hsplit: 9 column blocks, block k = w2T*w1[:,k], same both halves
        #  dual:   6 column blocks with per-half k (see _mm_dual):
        #          slot:   0     1     2     3     4     5
        #          lower:  k0    k1    k2    k3    k4    k5
        #          upper:  k6    k7    k8    k5    -     -
        if SCHEME == "dual":
            half_ks = ((0, 1, 2, 3, 4, 5), (6, 7, 8, 5))
            n_blocks = 6
        else:
            half_ks = (tuple(range(9)), tuple(range(9)))
            n_blocks = 9
        wf_tiles = {}
        for s, base, w1s in (("x", w2xT_s, w1x_s), ("y", w2yT_s, w1y_s), ("n", u2Tn, u1)):
            wf = wp.tile([2 * CIN, n_blocks * COUT], mdt, tag=f"wf{s}")
            for half in (0, 1):
                p0, p1 = half * CIN, (half + 1) * CIN
                for slot, k in enumerate(half_ks[half]):
                    nc.vector.tensor_scalar(
                        wf[p0:p1, slot * COUT : (slot + 1) * COUT],
                        base[p0:p1, :],
                        w1s[p0:p1, k : k + 1],
                        None,
                        mybir.AluOpType.mult,
                    )
            wf_tiles[s] = wf

        # ---- main compute ------------------------------------------------
        def main_body():
            for b, s, needs_log, evac in BRANCHES:
                wf = wf_tiles[s]
                img = imgp.tile([2 * CIN, img_rows, WP], mdt, tag="img")
                nc.sync.dma_start(out=img[0:CIN], in_=xin_d[b, 0])
                nc.sync.dma_start(out=img[CIN : 2 * CIN], in_=xin_d[b, 1])
                if needs_log:
                    nc.scalar.activation(
                        img[:, :, :],
                        img[:, :, :],
                        mybir.ActivationFunctionType.Ln,
                        bias=eps_b[:, 0:1],
                    )
                for tp in range(8):
                    ps = psp.tile([COUT, 8, W], F32, tag="ps")
                    if PROBE != "no_mm":
                        if SCHEME == "dual":
                            _mm_dual(nc, ps, wf, img, tp)
                        else:
                            _mm_hsplit(nc, ps, wf, img, tp)
                    if PROBE == "no_out":
                        continue
                    ot = obp.tile([COUT, 8, W], F32, tag="ot")
                    h0 = 8 * tp
                    if needs_log:
                        nc.scalar.activation(
                            ot[:, :, :],
                            ps[:, :, :],
                            mybir.ActivationFunctionType.Exp,
                            bias=zero_b[:, 0:1],
                        )
                    elif evac == "v":
                        nc.vector.tensor_copy(ot[:, :, :], ps[:, :, :])
                    else:
                        nc.scalar.activation(
                            ot[:, :, :], ps[:, :, :], mybir.ActivationFunctionType.Copy
                        )
                    nc.sync.dma_start(out=out_d[b, :, h0 : h0 + 8, :], in_=ot[:, :, :])

        if LOOP_ITERS:
            with tc.For_i(0, LOOP_ITERS, 1):
                main_body()
        else:
            main_body()


def _wfk(wf, k, half):
    p0, p1 = half * CIN, (half + 1) * CIN
    return wf[p0:p1, k * COUT : (k + 1) * COUT]


def _mm_dual(nc, ps, wf, img, tp):
    """out rows 8*tp..8*tp+7 from dual-copy image: partitions 0..63 hold the
    padded image A (rows 0..65), partitions 64..127 hold B with B[r]=A[r+2].

    6 matmuls per tile: 3x K=128 (offset pairs (-1,dw)+(+1,dw)), then the
    dh=0 row as K=64 matmuls -- (0,-1) on the lower row group packed with
    (0,+1) on the upper row group (concurrent), plus (0,0) on the lower."""
    h0 = 8 * tp
    n_mm = 6
    idx = [0]

    def step(lhsT, rhs):
        nc.tensor.matmul(
            ps[:, :, :], lhsT, rhs, start=(idx[0] == 0), stop=(idx[0] == n_mm - 1)
        )
        idx[0] += 1

    for dw in (-1, 0, 1):  # slots 0..2: K=128, lower k=dw+1, upper k=7+dw
        step(
            wf[:, (dw + 1) * COUT : (dw + 2) * COUT],
            img[:, h0 : h0 + 8, 1 + dw : 1 + dw + W],
        )
    # (0,-1) lower (slot3 low) ++ (0,+1) upper (slot3 high, B[h0-1]=A[h0+1])
    step(wf[0:CIN, 3 * COUT : 4 * COUT], img[0:CIN, h0 + 1 : h0 + 9, 0:W])
    if DH0_UPPER_PACK and tp > 0:
        step(
            wf[CIN : 2 * CIN, 3 * COUT : 4 * COUT],
            img[CIN : 2 * CIN, h0 - 1 : h0 + 7, 2 : 2 + W],
        )
    else:  # B row -1 unavailable (tp=0) or packing disabled: lower, slot 5
        step(wf[0:CIN, 5 * COUT : 6 * COUT], img[0:CIN, h0 + 1 : h0 + 9, 2 : 2 + W])
    # (0,0) lower (slot4 low)
    step(wf[0:CIN, 4 * COUT : 5 * COUT], img[0:CIN, h0 + 1 : h0 + 9, 1 : 1 + W])


def _mm_hsplit(nc, ps, wf, img, tp):
    """hsplit scheme: tile tp covers out rows 8*tp..+7; lower tiles (tp<4)
    read partitions 0..63, upper tiles read 64..127."""
    half = 0 if tp < 4 else 1
    p0, p1 = half * CIN, (half + 1) * CIN
    tpl = tp % 4
    for k in range(9):
        dh, dw = k // 3 - 1, k % 3 - 1
        r = 8 * tpl + 1 + dh + half  # lower: pad row - 0; upper: pad row - 31
        c0 = 1 + dw
        nc.tensor.matmul(
            ps[:, :, :],
            _wfk(wf, k, half),
            img[p0:p1, r : r + 8, c0 : c0 + W],
            start=(k == 0),
            stop=(k == 8),
        )


def build_program():
    key = (MM_DTYPE, SCHEME, LOOP_ITERS, DH0_UPPER_PACK, PROBE, LDW_OPT)
    if key in _PROG_CACHE:
        return _PROG_CACHE[key]
    img_rows = HP if SCHEME == "dual" else HS_ROWS
    nc = bacc.Bacc("TRN2", target_bir_lowering=False, debug=False)
    xin_d = nc.dram_tensor(
        "xin", [NBR, 2, CIN, img_rows, WP], _mm_dt(), kind="ExternalInput"
    ).ap()
    w1x_d = nc.dram_tensor("w1x", [CIN, 9], F32, kind="ExternalInput").ap()
    w1y_d = nc.dram_tensor("w1y", [CIN, 9], F32, kind="ExternalInput").ap()
    w2xT_d = nc.dram_tensor("w2xT", [CIN, COUT], F32, kind="ExternalInput").ap()
    w2yT_d = nc.dram_tensor("w2yT", [CIN, COUT], F32, kind="ExternalInput").ap()
    out_d = nc.dram_tensor("out", [NBR, COUT, H, W], F32, kind="ExternalOutput").ap()
    with tile.TileContext(nc) as tc:
        _emit(nc, tc, xin_d, w1x_d, w1y_d, w2xT_d, w2yT_d, out_d)
    nc.compile()
    _PROG_CACHE[key] = nc
    return nc


def marshal_inputs(x, w1x, w1y, w2x, w2y):
    """Host-side data marshaling: shard over batch, zero-pad, build the
    per-partition-half copies for the selected scheme."""
    ndt = _np_in_dt()
    x = np.asarray(x, dtype=np.float32)
    xp = np.zeros((B, NBR, CIN, HP, WP), np.float32)
    xp[:, :, :, 1 : H + 1, 1 : W + 1] = x
    if SCHEME == "dual":
        xin = np.zeros((B, NBR, 2, CIN, HP, WP), ndt)
        xin[:, :, 0] = xp.astype(ndt)
        xin[:, :, 1, :, 0 : HP - 2, :] = xp[:, :, :, 2:HP, :].astype(ndt)
    else:
        xin = np.empty((B, NBR, 2, CIN, HS_ROWS, WP), ndt)
        xin[:, :, 0] = xp[:, :, :, 0:HS_ROWS, :].astype(ndt)
        xin[:, :, 1] = xp[:, :, :, HP - HS_ROWS : HP, :].astype(ndt)
    w2xT = np.ascontiguousarray(np.asarray(w2x, np.float32).T)
    w2yT = np.ascontiguousarray(np.asarray(w2y, np.float32).T)
    w1x = np.ascontiguousarray(w1x, np.float32)
    w1y = np.ascontiguousarray(w1y, np.float32)
    return [
        {
            "xin": np.ascontiguousarray(xin[i]),
            "w1x": w1x,
            "w1y": w1y,
            "w2xT": w2xT,
            "w2yT": w2yT,
        }
        for i in range(B)
    ]


def kernel(x, w1x, w1y, w2x, w2y):
    global LAST_EXEC_TIME_NS, LAST_RESULTS
    nc = build_program()
    in_maps = marshal_inputs(x, w1x, w1y, w2x, w2y)
    res = bass_utils.run_bass_kernel_spmd(
        nc, in_maps, list(range(N_CORES)), trace=TRACE
    )
    LAST_EXEC_TIME_NS = res.exec_time_ns
    LAST_RESULTS = res
    out = np.stack([res.results[i]["out"] for i in range(N_CORES)], axis=0)
    return np.asarray(out, np.float32)



# revision 4
# speedup vs baseline: 1.9571x; 1.9571x over previous
"""Trainium2 Bass kernel for nn_ComplexConv2Deffangle4Dxy.

Reference math (per batch b, branch br):
    out[br] = pointwise(w2, depthwise3x3(w1, img[br]))   with zero padding P=1
      br=0 (rot): weights (w1n, w2n) where wn = (wx+wy)^2 / sum((wx+wy)^2)
      br=1 (abs): log-domain: exp(branch(log(img + EPS), w1n, w2n))
      br=2 (x):   weights (w1x, w2x)
      br=3 (y):   weights (w1y, w2y)

Kernel strategy (per NeuronCore, data-parallel over batch B=8 -> 8 cores):
  Fuse depthwise+pointwise into a single 3x3 conv whose weights are the
  outer product  Wf[o, c, k] = w2[o, c] * w1[c, k], computed as
  PSUM-accumulated matmuls over the 9 kernel offsets with
  lhsT = fused weights (K=Cin=64, M=Cout=128) and rhs = shifted image views.
  Images are zero-padded on the host (pure marshaling); for the abs branch
  Ln(x+EPS) maps the zero padding to log(EPS), matching the reference's
  pad-then-log order.  Weight normalization for the rot/abs branches is
  computed on device.

  Image layout ("hsplit"): SBUF partitions 0..63 hold padded rows 0..34,
  partitions 64..127 hold padded rows 31..65.  Output row-tiles 0..3 read
  the lower half (PE row groups 0-1), tiles 4..7 the upper half (row
  groups 2-3).

  Matmul issue order: tap-outer, tile-inner, alternating lower/upper so
  consecutive matmuls land on disjoint PE row groups and overlap in the
  array; all 8 PSUM banks hold live accumulators so weights stay loaded
  across the 4 tiles of each (tap, half).  bf16 operands enable split
  LDWEIGHTS (walrus --enable-ldw-opt) + fast weight load; outputs are
  DMA'd as bf16 and upcast on host (all within the rel-err budget).
"""

import sys

for _p in ("/opt/trn_rl_repo",):
    if _p not in sys.path:
        sys.path.insert(0, _p)

import ml_dtypes
import numpy as np

import concourse.bacc as bacc
import concourse.mybir as mybir
import concourse.tile as tile
from concourse import bass_utils

F32 = mybir.dt.float32
F32R = mybir.dt.float32r
BF16 = mybir.dt.bfloat16

EPS = 1e-6
N_CORES = 8
B, NBR, CIN, COUT, H, W = 8, 4, 64, 128, 64, 64
HP, WP = H + 2, W + 2          # host-padded image
HS_ROWS = 35                   # hsplit: padded rows per partition half

MM_DTYPE = "bf16"              # matmul input dtype: "f32r" | "f32" | "bf16"
OUT_BF16 = True                # DMA outputs as bf16, upcast to f32 on host
ISSUE = "ilv"                  # "ilv" (interleaved halves) | "seq" (baseline)
LOOP_ITERS = None              # benchmarking: device-side repeat count
PROBE = ""                     # "" | "no_out" (skip evac+out-DMA) | "no_mm"
TRACE = False
LAST_EXEC_TIME_NS = None
LAST_RESULTS = None

_PROG_CACHE = {}

# walrus's LDWEIGHTS optimization: NOT needed — bass already splits bf16
# matmuls into InstLdweights + InstMatmult at the BIR level (which is what
# lets the PE reorder window pull weight loads ahead).  Enabling the walrus
# flag on the pre-split form fails codegen ("InstLdweights is not compatible
# with LDW optimization"), so keep False.
LDW_OPT = False
_orig_run_command = bass_utils.run_command


def _patched_run_command(cmd, *a, **kw):
    if (
        LDW_OPT
        and MM_DTYPE == "bf16"
        and isinstance(cmd, list)
        and "--enable-ldw-opt=false" in cmd
    ):
        cmd = ["--enable-ldw-opt=true" if c == "--enable-ldw-opt=false" else c for c in cmd]
    return _orig_run_command(cmd, *a, **kw)


bass_utils.run_command = _patched_run_command
if getattr(bass_utils, "bir_verify_and_optimise", None) is not None:
    bass_utils.bir_verify_and_optimise.__globals__["run_command"] = _patched_run_command

BRANCHES = (  # (branch index, weight set, log-domain?)
    (2, "x", False),
    (3, "y", False),
    (0, "n", False),
    (1, "n", True),
)


def _mm_dt():
    return {"f32r": F32R, "f32": F32, "bf16": BF16}[MM_DTYPE]


def _np_in_dt():
    return ml_dtypes.bfloat16 if MM_DTYPE == "bf16" else np.float32


def _out_dt():
    return BF16 if OUT_BF16 else F32


def _emit(nc, tc, xin_d, w1x_d, w1y_d, w2xT_d, w2yT_d, out_d):
    mdt = _mm_dt()
    odt = _out_dt()
    with (
        tc.tile_pool(name="wp", bufs=1) as wp,
        tc.tile_pool(name="imgp", bufs=2) as imgp,
        tc.tile_pool(name="psp", bufs=8, space="PSUM") as psp,
        tc.tile_pool(name="obp", bufs=6) as obp,
    ):
        # ---- weight prep -------------------------------------------------
        # All weight/source tiles replicated into both partition halves so
        # both PE row groups see the same fused weights.
        w1x_s = wp.tile([2 * CIN, 9], F32, tag="w1x")
        w1y_s = wp.tile([2 * CIN, 9], F32, tag="w1y")
        w2xT_s = wp.tile([2 * CIN, COUT], F32, tag="w2xT")
        w2yT_s = wp.tile([2 * CIN, COUT], F32, tag="w2yT")
        for t, d in (
            (w1x_s, w1x_d),
            (w1y_s, w1y_d),
            (w2xT_s, w2xT_d),
            (w2yT_s, w2yT_d),
        ):
            nc.sync.dma_start(out=t[0:CIN], in_=d)
            nc.sync.dma_start(out=t[CIN : 2 * CIN], in_=d)

        ones_k = wp.tile([CIN, 1], F32, tag="ones_k")
        nc.vector.memset(ones_k[:, :], 1.0)
        ones_m = wp.tile([1, 2 * CIN], F32, tag="ones_m")
        nc.vector.memset(ones_m[:, :], 1.0)
        eps_b = wp.tile([2 * CIN, 1], F32, tag="eps_b")
        nc.vector.memset(eps_b[:, :], float(EPS))
        zero_b = wp.tile([COUT, 1], F32, tag="zero_b")
        nc.vector.memset(zero_b[:, :], 0.0)

        # u1 = (w1x + w1y)^2, u2T = ((w2x + w2y)^2)^T  (both partition halves)
        u1 = wp.tile([2 * CIN, 9], F32, tag="u1")
        nc.vector.tensor_add(u1[:, :], w1x_s[:, :], w1y_s[:, :])
        nc.vector.tensor_mul(u1[:, :], u1[:, :], u1[:, :])
        u2T = wp.tile([2 * CIN, COUT], F32, tag="u2T")
        nc.vector.tensor_add(u2T[:, :], w2xT_s[:, :], w2yT_s[:, :])
        nc.vector.tensor_mul(u2T[:, :], u2T[:, :], u2T[:, :])

        # S1 = sum(u1), S2 = sum(u2) via ones-matmul + free-dim reduce
        s1v = psp.tile([1, 9], F32, tag="ps")
        nc.tensor.matmul(s1v[:, :], ones_k[:, :], u1[0:CIN, :], start=True, stop=True)
        s2v = psp.tile([1, COUT], F32, tag="ps")
        nc.tensor.matmul(s2v[:, :], ones_k[:, :], u2T[0:CIN, :], start=True, stop=True)
        s1 = wp.tile([1, 1], F32, tag="s1")
        nc.vector.tensor_reduce(
            s1[:, :], s1v[:, :], axis=mybir.AxisListType.X, op=mybir.AluOpType.add
        )
        s2 = wp.tile([1, 1], F32, tag="s2")
        nc.vector.tensor_reduce(
            s2[:, :], s2v[:, :], axis=mybir.AxisListType.X, op=mybir.AluOpType.add
        )
        inv = wp.tile([1, 1], F32, tag="inv")
        nc.vector.tensor_mul(inv[:, :], s1[:, :], s2[:, :])
        nc.vector.reciprocal(inv[:, :], inv[:, :])
        # broadcast 1/(S1*S2) to all 128 partitions
        invb_ps = psp.tile([2 * CIN, 1], F32, tag="ps")
        nc.tensor.matmul(invb_ps[:, :], ones_m[:, :], inv[:, :], start=True, stop=True)
        invb = wp.tile([2 * CIN, 1], F32, tag="invb")
        nc.vector.tensor_copy(invb[:, :], invb_ps[:, :])
        # u2T_n = u2T / (S1*S2): both normalizations in one fold
        u2Tn = wp.tile([2 * CIN, COUT], F32, tag="u2Tn")
        nc.vector.tensor_scalar(
            u2Tn[:, :], u2T[:, :], invb[:, 0:1], None, mybir.AluOpType.mult
        )

        # fused weight tiles: 9 column blocks, block k = w2T * w1[:, k],
        # identical in both partition halves (built in one op across 128
        # partitions since the scalar operand is partition-local).
        wf_tiles = {}
        for s, base, w1s in (("x", w2xT_s, w1x_s), ("y", w2yT_s, w1y_s), ("n", u2Tn, u1)):
            wf = wp.tile([2 * CIN, 9 * COUT], mdt, tag=f"wf{s}")
            for k in range(9):
                nc.vector.tensor_scalar(
                    wf[:, k * COUT : (k + 1) * COUT],
                    base[:, :],
                    w1s[:, k : k + 1],
                    None,
                    mybir.AluOpType.mult,
                )
            wf_tiles[s] = wf

        # ---- main compute ------------------------------------------------
        def main_body():
            for b, s, needs_log in BRANCHES:
                wf = wf_tiles[s]
                img = imgp.tile([2 * CIN, HS_ROWS, WP], mdt, tag="img")
                nc.sync.dma_start(out=img[0:CIN], in_=xin_d[b, 0])
                nc.sync.dma_start(out=img[CIN : 2 * CIN], in_=xin_d[b, 1])
                if needs_log:
                    nc.scalar.activation(
                        img[:, :, :],
                        img[:, :, :],
                        mybir.ActivationFunctionType.Ln,
                        bias=eps_b[:, 0:1],
                    )
                ps = [
                    psp.tile([COUT, 8, W], F32, tag="ps", name=f"ps{t}")
                    for t in range(8)
                ]
                if PROBE != "no_mm":
                    if ISSUE == "ilv":
                        _mm_ilv(nc, ps, wf, img)
                    else:
                        _mm_seq(nc, ps, wf, img)
                if PROBE == "no_out":
                    continue
                for t in range(8):
                    ot = obp.tile([COUT, 8, W], odt, tag="ot")
                    h0 = 8 * t
                    if needs_log:
                        nc.scalar.activation(
                            ot[:, :, :],
                            ps[t][:, :, :],
                            mybir.ActivationFunctionType.Exp,
                            bias=zero_b[:, 0:1],
                        )
                    elif t % 2 == 0:
                        nc.vector.tensor_copy(ot[:, :, :], ps[t][:, :, :])
                    else:
                        nc.scalar.activation(
                            ot[:, :, :], ps[t][:, :, :], mybir.ActivationFunctionType.Copy
                        )
                    nc.sync.dma_start(out=out_d[b, :, h0 : h0 + 8, :], in_=ot[:, :, :])

        if LOOP_ITERS:
            with tc.For_i(0, LOOP_ITERS, 1):
                main_body()
        else:
            main_body()


def _rhs(img, half, tpl, k):
    """Shifted image view for out-row-tile (half, tpl) and tap k."""
    dh, dw = k // 3 - 1, k % 3 - 1
    p0, p1 = half * CIN, (half + 1) * CIN
    r = 8 * tpl + 1 + dh + half  # lower: pad row - 0; upper: pad row - 31
    c0 = 1 + dw
    return img[p0:p1, r : r + 8, c0 : c0 + W]


def _wfk(wf, k, half):
    p0, p1 = half * CIN, (half + 1) * CIN
    return wf[p0:p1, k * COUT : (k + 1) * COUT]


def _mm_ilv(nc, ps, wf, img):
    """Tap-outer, tile-inner, alternating lower/upper row groups.

    Consecutive matmuls target disjoint PE row groups (tile_position derives
    from lhsT base_partition: 0 vs 64) and distinct PSUM banks, so they run
    concurrently in the array; within a (tap, half) the 4 tiles share one
    weight load."""
    for k in range(9):
        st, sp = k == 0, k == 8
        for tpl in range(4):
            nc.tensor.matmul(
                ps[tpl][:, :, :],
                _wfk(wf, k, 0),
                _rhs(img, 0, tpl, k),
                start=st,
                stop=sp,
                skip_group_check=True,
            )
            nc.tensor.matmul(
                ps[tpl + 4][:, :, :],
                _wfk(wf, k, 1),
                _rhs(img, 1, tpl, k),
                start=st,
                stop=sp,
                skip_group_check=True,
            )


def _mm_seq(nc, ps, wf, img):
    """Baseline order: tile-outer, tap-inner (each tile's 9 taps serial)."""
    for t in range(8):
        half, tpl = (0, t) if t < 4 else (1, t - 4)
        for k in range(9):
            nc.tensor.matmul(
                ps[t][:, :, :],
                _wfk(wf, k, half),
                _rhs(img, half, tpl, k),
                start=(k == 0),
                stop=(k == 8),
                skip_group_check=True,
            )


def build_program():
    key = (MM_DTYPE, ISSUE, OUT_BF16, LOOP_ITERS, PROBE, LDW_OPT)
    if key in _PROG_CACHE:
        return _PROG_CACHE[key]
    nc = bacc.Bacc("TRN2", target_bir_lowering=False, debug=False)
    xin_d = nc.dram_tensor(
        "xin", [NBR, 2, CIN, HS_ROWS, WP], _mm_dt(), kind="ExternalInput"
    ).ap()
    w1x_d = nc.dram_tensor("w1x", [CIN, 9], F32, kind="ExternalInput").ap()
    w1y_d = nc.dram_tensor("w1y", [CIN, 9], F32, kind="ExternalInput").ap()
    w2xT_d = nc.dram_tensor("w2xT", [CIN, COUT], F32, kind="ExternalInput").ap()
    w2yT_d = nc.dram_tensor("w2yT", [CIN, COUT], F32, kind="ExternalInput").ap()
    out_d = nc.dram_tensor("out", [NBR, COUT, H, W], _out_dt(), kind="ExternalOutput").ap()
    with tile.TileContext(nc) as tc:
        _emit(nc, tc, xin_d, w1x_d, w1y_d, w2xT_d, w2yT_d, out_d)
    nc.compile()
    _PROG_CACHE[key] = nc
    return nc


def marshal_inputs(x, w1x, w1y, w2x, w2y):
    """Host-side data marshaling: shard over batch, zero-pad, build the
    per-partition-half copies (hsplit layout)."""
    ndt = _np_in_dt()
    x = np.asarray(x, dtype=np.float32)
    xp = np.zeros((B, NBR, CIN, HP, WP), np.float32)
    xp[:, :, :, 1 : H + 1, 1 : W + 1] = x
    xin = np.empty((B, NBR, 2, CIN, HS_ROWS, WP), ndt)
    xin[:, :, 0] = xp[:, :, :, 0:HS_ROWS, :].astype(ndt)
    xin[:, :, 1] = xp[:, :, :, HP - HS_ROWS : HP, :].astype(ndt)
    w2xT = np.ascontiguousarray(np.asarray(w2x, np.float32).T)
    w2yT = np.ascontiguousarray(np.asarray(w2y, np.float32).T)
    w1x = np.ascontiguousarray(w1x, np.float32)
    w1y = np.ascontiguousarray(w1y, np.float32)
    return [
        {
            "xin": np.ascontiguousarray(xin[i]),
            "w1x": w1x,
            "w1y": w1y,
            "w2xT": w2xT,
            "w2yT": w2yT,
        }
        for i in range(B)
    ]


def kernel(x, w1x, w1y, w2x, w2y):
    global LAST_EXEC_TIME_NS, LAST_RESULTS
    nc = build_program()
    in_maps = marshal_inputs(x, w1x, w1y, w2x, w2y)
    res = bass_utils.run_bass_kernel_spmd(
        nc, in_maps, list(range(N_CORES)), trace=TRACE
    )
    LAST_EXEC_TIME_NS = res.exec_time_ns
    LAST_RESULTS = res
    out = np.stack([res.results[i]["out"] for i in range(N_CORES)], axis=0)
    return np.asarray(out, np.float32)
